# revision 1
# baseline (speedup 1.0000x reference)
"""ChebyASPIRE spectral filter on 8 TRN2 NeuronCores.

Algorithm (Gram-matrix formulation):
  phase 1: Z = X^T X  (4096x4096), column-sharded: core i computes
           Z[:, i*512:(i+1)*512] from a full stream of X (fp16 operands,
           fp32 PSUM accumulate), kept in SBUF as fp16.
  phase 2: Chebyshev recurrence t_k = 2*Zs t_{k-1} - t_{k-2} with
           Zs = (Z - t_mid I)/t_half, applied to V = R^T (4096x256).
           Row-sharded: core i computes rows [i*512, (i+1)*512) of each
           t_k using lhsT = Z[:, ib] (== Z[ib, :]^T by symmetry), then
           AllGathers the new t shard so every core has the full t for
           the next step.  The batch (256 query users) is split into two
           halves that advance in lockstep but alternate on the PE, so
           each half's AllGather hides under the other half's matmuls.
           Recurrence state and accumulator stay fp32.

Inputs come in full; sharding/layout prep happens on host.  Scalars
(t_mid, t_half, coeffs) are baked into the program as immediates; the
program is rebuilt (and NEFF-cached) per distinct scalar set.
"""
import sys

sys.path.insert(0, "/opt/trn_rl_repo")

import numpy as np

M, N, B = 8192, 4096, 256
NC = 8
CB = N // NC          # 512 columns/rows per core
DEG = 20              # Chebyshev degree (21 coeffs)
KT1 = M // 128        # 64 k-tiles in phase 1
MP1 = N // 128        # 32 m-passes in phase 1
KT2 = N // 128        # 32 k-tiles in phase 2
MS2 = CB // 128       # 4 m-subs in phase 2
NH = 2                # batch halves in phase 2
BH = B // NH          # 128 columns per half

_BUILD_CACHE = {}


def _build(scalars):
    """Build the SPMD Bass program for the given (t_mid, t_half, *coeffs)."""
    from concourse import bacc, tile, mybir

    tm, th = scalars[0], scalars[1]
    c = scalars[2:]
    f16 = mybir.dt.float16
    f32 = mybir.dt.float32
    mult = mybir.AluOpType.mult
    add = mybir.AluOpType.add
    sub = mybir.AluOpType.subtract

    nc = bacc.Bacc("TRN2", target_bir_lowering=False, debug=False,
                   num_devices=NC)
    Xh = nc.dram_tensor("X16", [M, N], f16, kind="ExternalInput")
    Xb = nc.dram_tensor("Xblk16", [M, CB], f16, kind="ExternalInput")
    Vh = nc.dram_tensor("V16", [N, B], f16, kind="ExternalInput")
    Vb = nc.dram_tensor("Vblk32", [CB, B], f32, kind="ExternalInput")
    acc_out = nc.dram_tensor("acc_out", [CB, B], f32, kind="ExternalOutput")

    RCH = 4                      # k-tiles per rhs_res chunk (phase 1)
    LCH = 16                     # k-tiles per lh chunk (phase 1)
    with tile.TileContext(nc) as tc:
        with (
            tc.tile_pool(name="persist", bufs=1) as persist,
            tc.tile_pool(name="lstream", bufs=2) as lstream,
            tc.tile_pool(name="rhsp", bufs=2) as rhsp,
            tc.tile_pool(name="dve", bufs=4) as dvep,
            tc.tile_pool(name="stagep", bufs=2) as stagep,
            tc.tile_pool(name="ps1", bufs=2, space="PSUM") as ps1,
            tc.tile_pool(name="ps2", bufs=6, space="PSUM") as ps2p,
            tc.tile_pool(name="dram", bufs=1, space="DRAM") as dram,
        ):
            # ---------------- phase 1: Z[:, ib] = X^T X[:, ib] -------------
            # resident rhs X[:, ib], chunked so matmuls start early
            rhs_res = [persist.tile([128, RCH, CB], f16, name=f"rhs_res{cc}")
                       for cc in range(KT1 // RCH)]
            Xb3 = Xb[:, :].rearrange("(kk p) cb -> p kk cb", p=128)
            for cc in range(KT1 // RCH):
                nc.sync.dma_start(rhs_res[cc][:],
                                  Xb3[:, cc * RCH:(cc + 1) * RCH, :])

            zk = [persist.tile([128, CB], f16, name=f"zk{i}")
                  for i in range(KT2)]

            for mp in range(MP1):
                lhs = [lstream.tile([128, LCH, 128], f16, name=f"lh{h}")
                       for h in range(KT1 // LCH)]
                Xm3 = (Xh[:, mp * 128:(mp + 1) * 128]
                       .rearrange("(kk p) mc -> p kk mc", p=128))
                for h in range(KT1 // LCH):
                    nc.sync.dma_start(lhs[h][:],
                                      Xm3[:, h * LCH:(h + 1) * LCH, :])
                zps = ps1.tile([128, CB], f32, name="zps")
                for kk in range(KT1):
                    nc.tensor.matmul(
                        zps[:],
                        lhs[kk // LCH][:, kk % LCH, :],
                        rhs_res[kk // RCH][:, kk % RCH, :],
                        start=(kk == 0), stop=(kk == KT1 - 1))
                nc.vector.tensor_copy(zk[mp][:], zps[:])

            # ---------------- phase 2: Chebyshev recurrence ----------------
            # per-half fp32 state shards (128 part x [4 m-subs x 128])
            tstate = [[persist.tile([128, MS2, BH], f32, name=f"tst{h}_{i}")
                       for i in range(3)] for h in range(NH)]
            acc = [persist.tile([128, MS2, BH], f32, name=f"acc{h}")
                   for h in range(NH)]
            zero = persist.tile([128, BH], f32, name="zero")
            nc.any.memset(zero[:], 0.0)
            Vb3 = Vb[:, :].rearrange("(ms p) b -> p ms b", p=128)
            for h in range(NH):
                nc.sync.dma_start(tstate[h][0][:],
                                  Vb3[:, :, h * BH:(h + 1) * BH])

            Vh3 = Vh[:, :].rearrange("(kk p) b -> p kk b", p=128)
            agout = [[None] * NH for _ in range(DEG)]

            for s in range(1, DEG + 1):
                for h in range(NH):
                    # rhs: full t_{s-1} half (4096 x 128) fp16, k-packed
                    rh = rhsp.tile([128, KT2, BH], f16, name=f"rh{h}")
                    RC = KT2 // 4
                    if s == 1:
                        for ch in range(4):
                            nc.sync.dma_start(
                                rh[:, ch * RC:(ch + 1) * RC, :],
                                Vh3[:, ch * RC:(ch + 1) * RC,
                                    h * BH:(h + 1) * BH])
                    else:
                        src = (agout[s - 2][h][:, :]
                               .rearrange("(kk p) b -> p kk b", p=128))
                        for ch in range(4):
                            nc.sync.dma_start(
                                rh[:, ch * RC:(ch + 1) * RC, :],
                                src[:, ch * RC:(ch + 1) * RC, :])

                    Tc = tstate[h][(s - 1) % 3]
                    Tp = tstate[h][(s - 2) % 3] if s >= 2 else None
                    Tn = tstate[h][s % 3]
                    ach = acc[h]
                    if s < DEG:
                        stage = stagep.tile([128, MS2, BH], f16,
                                            name=f"stage{h}")
                        agin = dram.tile([CB, BH], f16, name=f"agin{s}_{h}")
                        agin3 = agin[:, :].rearrange("(ms p) b -> p ms b",
                                                     p=128)

                    for ms in range(MS2):
                        wps = ps2p.tile([128, BH], f32, name="wps")
                        for kk in range(KT2):
                            nc.tensor.matmul(
                                wps[:],
                                zk[kk][:, ms * 128:(ms + 1) * 128],
                                rh[:, kk, :],
                                start=(kk == 0), stop=(kk == KT2 - 1))

                        u = dvep.tile([128, BH], f32, name="u")
                        # u = W - tm * Tc
                        nc.vector.scalar_tensor_tensor(
                            u[:], Tc[:, ms, :], -tm, wps[:],
                            op0=mult, op1=add)
                        if s == 1:
                            # T1 = u / th ;  acc = c0*V + c1*T1
                            nc.vector.scalar_tensor_tensor(
                                Tn[:, ms, :], u[:], 1.0 / th, zero[:],
                                op0=mult, op1=sub)
                            nc.vector.tensor_scalar_mul(
                                ach[:, ms, :], Tc[:, ms, :], c[0])
                            nc.vector.scalar_tensor_tensor(
                                ach[:, ms, :], Tn[:, ms, :], c[1],
                                ach[:, ms, :], op0=mult, op1=add)
                        else:
                            # Tn = (2/th)*u - Tp ; acc += c_s * Tn
                            nc.vector.scalar_tensor_tensor(
                                Tn[:, ms, :], u[:], 2.0 / th, Tp[:, ms, :],
                                op0=mult, op1=sub)
                        if s < DEG:
                            nc.vector.tensor_copy(stage[:, ms, :],
                                                  Tn[:, ms, :])
                        if s > 1:
                            nc.vector.scalar_tensor_tensor(
                                ach[:, ms, :], Tn[:, ms, :], c[s],
                                ach[:, ms, :], op0=mult, op1=add)

                    if s < DEG:
                        nc.sync.dma_start(agin3[:], stage[:])
                        agout[s - 1][h] = dram.tile(
                            [N, BH], f16, addr_space="Shared",
                            name=f"agout{s}_{h}")
                        nc.gpsimd.collective_compute(
                            "AllGather",
                            mybir.AluOpType.bypass,
                            replica_groups=[list(range(NC))],
                            ins=[agin[:]],
                            outs=[agout[s - 1][h][:]],
                        )

            out3 = acc_out[:, :].rearrange("(ms p) b -> p ms b", p=128)
            for h in range(NH):
                nc.sync.dma_start(out3[:, :, h * BH:(h + 1) * BH],
                                  acc[h][:])

    nc.finalize()
    return nc


def _get_program(scalars):
    key = tuple(np.asarray(scalars, np.float64).tolist())
    if key not in _BUILD_CACHE:
        _BUILD_CACHE[key] = _build(key)
    return _BUILD_CACHE[key]


def _run(X, R, coeffs, t_mid, t_half, trace=False):
    from concourse.bass_utils import run_bass_kernel_spmd

    X = np.ascontiguousarray(np.asarray(X, np.float32))
    R = np.ascontiguousarray(np.asarray(R, np.float32))
    coeffs = np.asarray(coeffs, np.float32)
    tm = float(np.asarray(t_mid).reshape(-1)[0])
    th = float(np.asarray(t_half).reshape(-1)[0])

    nc = _get_program((tm, th, *[float(v) for v in coeffs]))

    X16 = X.astype(np.float16)
    V32 = np.ascontiguousarray(R.T.astype(np.float32))   # (N, B)
    V16 = V32.astype(np.float16)

    in_maps = []
    for i in range(NC):
        ib = slice(i * CB, (i + 1) * CB)
        in_maps.append({
            "X16": X16,
            "Xblk16": np.ascontiguousarray(X16[:, ib]),
            "V16": V16,
            "Vblk32": np.ascontiguousarray(V32[ib, :]),
        })

    res = run_bass_kernel_spmd(nc, in_maps, core_ids=list(range(NC)),
                               trace=trace)

    out = np.empty((B, N), np.float32)
    for i in range(NC):
        out[:, i * CB:(i + 1) * CB] = res.results[i]["acc_out"].T
    return out, res


def kernel(X, R, coeffs, t_mid, t_half):
    out, _ = _run(X, R, coeffs, t_mid, t_half, trace=False)
    return out



# revision 2
# speedup vs baseline: 1.1630x; 1.1630x over previous
"""ChebyASPIRE spectral filter on 8 TRN2 NeuronCores.

Algorithm (Gram-matrix formulation):
  phase 1: Z = X^T X  (4096x4096), column-sharded: core i computes
           Z[:, i*512:(i+1)*512] from a full stream of X.  Operands are
           fp8-e4m3 (X pre-scaled by 32 on host) using DoubleRow perf
           mode (2 fp8 macs/cell/cycle); PSUM accumulates fp32; the
           1/1024 descale folds into the PSUM->SBUF copy.  Z kept fp16.
  phase 2: Chebyshev recurrence t_k = 2*Zs t_{k-1} - t_{k-2} with
           Zs = (Z - t_mid I)/t_half, applied to V = R^T (4096x256).
           Row-sharded: core i computes rows [i*512, (i+1)*512) of each
           t_k using lhsT = Z[:, ib] (== Z[ib, :]^T by symmetry), then
           AllGathers the new t shard so every core has the full t for
           the next step.  The gathered state travels as fp8-e4m3
           (|t| <= 1) to halve collective wire bytes; the local
           recurrence state and accumulator stay fp32.  The batch is
           split into two halves that advance in lockstep but alternate
           on the PE, so each half's AllGather hides under the other
           half's matmuls.

Inputs come in full; sharding/layout prep happens on host.  Scalars
(t_mid, t_half, coeffs) are baked into the program as immediates; the
program is rebuilt (and NEFF-cached) per distinct scalar set.
"""
import sys

sys.path.insert(0, "/opt/trn_rl_repo")

import numpy as np

M, N, B = 8192, 4096, 256
NC = 8
CB = N // NC          # 512 columns/rows per core
DEG = 20              # Chebyshev degree (21 coeffs)
KT1 = M // 128        # 64 k-tiles in phase 1
KP1 = KT1 // 2        # 32 DoubleRow k-pairs in phase 1
MP1 = N // 128        # 32 m-passes in phase 1
KT2 = N // 128        # 32 k-tiles in phase 2
MS2 = CB // 128       # 4 m-subs in phase 2
NH = 2                # batch halves in phase 2
BH = B // NH          # 128 columns per half
XSCALE = 32.0         # host-side fp8 pre-scale on X
ZDESCALE = 1.0 / (XSCALE * XSCALE)

_BUILD_CACHE = {}


def _build(scalars):
    """Build the SPMD Bass program for the given (t_mid, t_half, *coeffs)."""
    from concourse import bacc, tile, mybir

    tm, th = scalars[0], scalars[1]
    c = scalars[2:]
    f8 = mybir.dt.float8e4
    f16 = mybir.dt.float16
    f32 = mybir.dt.float32
    mult = mybir.AluOpType.mult
    add = mybir.AluOpType.add
    sub = mybir.AluOpType.subtract
    DR = mybir.MatmulPerfMode.DoubleRow

    nc = bacc.Bacc("TRN2", target_bir_lowering=False, debug=False,
                   num_devices=NC)
    Xh = nc.dram_tensor("X8", [M, N], f8, kind="ExternalInput")
    Xb = nc.dram_tensor("Xblk8", [M, CB], f8, kind="ExternalInput")
    Vh = nc.dram_tensor("V8", [N, B], f8, kind="ExternalInput")
    Vb = nc.dram_tensor("Vblk32", [CB, B], f32, kind="ExternalInput")
    acc_out = nc.dram_tensor("acc_out", [CB, B], f32, kind="ExternalOutput")

    RCH = 4                      # k-tiles per rhs_res chunk (phase 1)
    LCH = 16                     # k-tiles per lh chunk (phase 1)
    with tile.TileContext(nc) as tc:
        with (
            tc.tile_pool(name="persist", bufs=1) as persist,
            tc.tile_pool(name="lstream", bufs=2) as lstream,
            tc.tile_pool(name="rhsp", bufs=2) as rhsp,
            tc.tile_pool(name="dve", bufs=4) as dvep,
            tc.tile_pool(name="stagep", bufs=2) as stagep,
            tc.tile_pool(name="ps1", bufs=2, space="PSUM") as ps1,
            tc.tile_pool(name="ps2", bufs=6, space="PSUM") as ps2p,
            tc.tile_pool(name="dram", bufs=1, space="DRAM") as dram,
        ):
            # ---------------- phase 1: Z[:, ib] = X^T X[:, ib] -------------
            # resident rhs X[:, ib], chunked so matmuls start early
            rhs_res = [persist.tile([128, RCH, CB], f8, name=f"rhs_res{cc}")
                       for cc in range(KT1 // RCH)]
            Xb3 = Xb[:, :].rearrange("(kk p) cb -> p kk cb", p=128)
            for cc in range(KT1 // RCH):
                nc.sync.dma_start(rhs_res[cc][:],
                                  Xb3[:, cc * RCH:(cc + 1) * RCH, :])

            # phase-2 state init + step-1 rhs: tiny DMAs, issue early so
            # they don't trail the 33 MB X stream.
            tstate = [[persist.tile([128, MS2, BH], f32, name=f"tst{h}_{i}")
                       for i in range(3)] for h in range(NH)]
            acc = [persist.tile([128, MS2, BH], f32, name=f"acc{h}")
                   for h in range(NH)]
            zero = persist.tile([128, BH], f32, name="zero")
            nc.any.memset(zero[:], 0.0)
            Vb3 = Vb[:, :].rearrange("(ms p) b -> p ms b", p=128)
            Vh3 = Vh[:, :].rearrange("(kk p) b -> p kk b", p=128)
            rh1 = []
            for h in range(NH):
                nc.sync.dma_start(tstate[h][0][:],
                                  Vb3[:, :, h * BH:(h + 1) * BH])
                rh = rhsp.tile([128, KT2, BH], f8, name=f"rh1_{h}")
                nc.sync.dma_start(rh[:],
                                  Vh3[:, :, h * BH:(h + 1) * BH])
                rh1.append(rh)

            zk = [persist.tile([128, CB], f16, name=f"zk{i}")
                  for i in range(KT2)]

            for mp in range(MP1):
                lhs = [lstream.tile([128, LCH, 128], f8, name=f"lh{h}")
                       for h in range(KT1 // LCH)]
                Xm3 = (Xh[:, mp * 128:(mp + 1) * 128]
                       .rearrange("(kk p) mc -> p kk mc", p=128))
                for h in range(KT1 // LCH):
                    nc.sync.dma_start(lhs[h][:],
                                      Xm3[:, h * LCH:(h + 1) * LCH, :])
                zps = ps1.tile([128, CB], f32, name="zps")
                for kp in range(KP1):
                    kk = 2 * kp
                    nc.tensor.matmul(
                        zps[:],
                        lhs[kk // LCH][:, kk % LCH:kk % LCH + 2, :],
                        rhs_res[kk // RCH][:, kk % RCH:kk % RCH + 2, :],
                        start=(kp == 0), stop=(kp == KP1 - 1),
                        perf_mode=DR)
                nc.vector.tensor_scalar_mul(zk[mp][:], zps[:], ZDESCALE)

            # ---------------- phase 2: Chebyshev recurrence ----------------
            agout = [[None] * NH for _ in range(DEG)]

            for s in range(1, DEG + 1):
                for h in range(NH):
                    # rhs: full t_{s-1} half (4096 x 128) fp8, k-packed
                    if s == 1:
                        rh = rh1[h]
                    else:
                        rh = rhsp.tile([128, KT2, BH], f8, name=f"rh{h}")
                        src = (agout[s - 2][h][:, :]
                               .rearrange("(kk p) b -> p kk b", p=128))
                        RC = KT2 // 4
                        for ch in range(4):
                            nc.sync.dma_start(
                                rh[:, ch * RC:(ch + 1) * RC, :],
                                src[:, ch * RC:(ch + 1) * RC, :])

                    Tc = tstate[h][(s - 1) % 3]
                    Tp = tstate[h][(s - 2) % 3] if s >= 2 else None
                    Tn = tstate[h][s % 3]
                    ach = acc[h]
                    if s < DEG:
                        stage = stagep.tile([128, MS2, BH], f8,
                                            name=f"stage{h}")
                        agin = dram.tile([CB, BH], f8, name=f"agin{s}_{h}")
                        agin3 = agin[:, :].rearrange("(ms p) b -> p ms b",
                                                     p=128)

                    for ms in range(MS2):
                        wps = ps2p.tile([128, BH], f32, name="wps")
                        for kk in range(KT2):
                            nc.tensor.matmul(
                                wps[:],
                                zk[kk][:, ms * 128:(ms + 1) * 128],
                                rh[:, kk, :],
                                start=(kk == 0), stop=(kk == KT2 - 1))

                        u = dvep.tile([128, BH], f32, name="u")
                        # u = W - tm * Tc
                        nc.vector.scalar_tensor_tensor(
                            u[:], Tc[:, ms, :], -tm, wps[:],
                            op0=mult, op1=add)
                        if s == 1:
                            # T1 = u / th ;  acc = c0*V + c1*T1
                            nc.vector.scalar_tensor_tensor(
                                Tn[:, ms, :], u[:], 1.0 / th, zero[:],
                                op0=mult, op1=sub)
                            nc.vector.tensor_scalar_mul(
                                ach[:, ms, :], Tc[:, ms, :], c[0])
                            nc.vector.scalar_tensor_tensor(
                                ach[:, ms, :], Tn[:, ms, :], c[1],
                                ach[:, ms, :], op0=mult, op1=add)
                        else:
                            # Tn = (2/th)*u - Tp ; acc += c_s * Tn
                            nc.vector.scalar_tensor_tensor(
                                Tn[:, ms, :], u[:], 2.0 / th, Tp[:, ms, :],
                                op0=mult, op1=sub)
                        if s < DEG:
                            nc.vector.tensor_copy(stage[:, ms, :],
                                                  Tn[:, ms, :])
                        if s > 1:
                            nc.vector.scalar_tensor_tensor(
                                ach[:, ms, :], Tn[:, ms, :], c[s],
                                ach[:, ms, :], op0=mult, op1=add)

                    if s < DEG:
                        nc.sync.dma_start(agin3[:], stage[:])
                        agout[s - 1][h] = dram.tile(
                            [N, BH], f8, addr_space="Shared",
                            name=f"agout{s}_{h}")
                        nc.gpsimd.collective_compute(
                            "AllGather",
                            mybir.AluOpType.bypass,
                            replica_groups=[list(range(NC))],
                            ins=[agin[:]],
                            outs=[agout[s - 1][h][:]],
                        )

            out3 = acc_out[:, :].rearrange("(ms p) b -> p ms b", p=128)
            for h in range(NH):
                nc.sync.dma_start(out3[:, :, h * BH:(h + 1) * BH],
                                  acc[h][:])

    nc.finalize()
    return nc


def _get_program(scalars):
    key = tuple(np.asarray(scalars, np.float64).tolist())
    if key not in _BUILD_CACHE:
        _BUILD_CACHE[key] = _build(key)
    return _BUILD_CACHE[key]


def _run(X, R, coeffs, t_mid, t_half, trace=False):
    import ml_dtypes
    from concourse.bass_utils import run_bass_kernel_spmd

    X = np.ascontiguousarray(np.asarray(X, np.float32))
    R = np.ascontiguousarray(np.asarray(R, np.float32))
    coeffs = np.asarray(coeffs, np.float32)
    tm = float(np.asarray(t_mid).reshape(-1)[0])
    th = float(np.asarray(t_half).reshape(-1)[0])

    nc = _get_program((tm, th, *[float(v) for v in coeffs]))

    f8np = ml_dtypes.float8_e4m3
    X8 = (X * XSCALE).astype(f8np)
    V32 = np.ascontiguousarray(R.T.astype(np.float32))   # (N, B)
    V8 = V32.astype(f8np)

    in_maps = []
    for i in range(NC):
        ib = slice(i * CB, (i + 1) * CB)
        in_maps.append({
            "X8": X8,
            "Xblk8": np.ascontiguousarray(X8[:, ib]),
            "V8": V8,
            "Vblk32": np.ascontiguousarray(V32[ib, :]),
        })

    res = run_bass_kernel_spmd(nc, in_maps, core_ids=list(range(NC)),
                               trace=trace)

    out = np.empty((B, N), np.float32)
    for i in range(NC):
        out[:, i * CB:(i + 1) * CB] = res.results[i]["acc_out"].T
    return out, res


def kernel(X, R, coeffs, t_mid, t_half):
    out, _ = _run(X, R, coeffs, t_mid, t_half, trace=False)
    return out
